# revision 10
# baseline (speedup 1.0000x reference)
"""Attention4DDownsample forward on 8 Trainium2 NeuronCores (Bass/Tile).

Data-parallel: batch 2048 -> 8 shards of 256 elements, one per core; all
parameters replicated. Per core the work is tiled in groups of NB=8 batch
elements.

Math folding done on the host (numpy):
  - every inference BN folds into the preceding conv's weight/bias
  - the LGQuery depthwise 3x3 + avgpool + 1x1 projection folds into 13
    "tap" matmuls over x (9 depthwise taps + 4 pool taps), so the whole
    query path runs on the PE array
  - the K-branch bias drops entirely (softmax is shift-invariant in n)
  - attention scale (kd^-0.5) folds into the Q weights/bias

On-chip structure per tile of 8 elements (all matmul I/O bf16, fp32 PSUM):
  - K/V/Q-tap convs: weight-stationary matmuls over x [c, (b, n)]
  - QK^T: per element one matmul with a block-diagonal stationary q_bd
    (built by one batched DVE multiply against a constant head mask)
  - softmax: DVE bias-add + ACT exp + DVE row-sum/reciprocal/scale,
    batched over the tile; no max-subtraction (logits are small)
  - P^T and V^T via the DMA crossbar transpose (dma_start(transpose=True))
    out of zero-padded 128-wide per-element panels
  - A@V: per (element, head-chunk) one matmul, vT stationary; the two
    half-garbage output halves are selected during eviction
  - v_local depthwise 3x3 runs as 9 clipped scalar_tensor_tensor taps on
    DVE/GpSimd; bias fused into the first (full-range) tap
  - proj: weight-stationary matmuls; BN bias fused in the ACT eviction
"""

import sys
for _p in ("/opt/trn_rl_repo", "/root/.axon_site", "/root/.axon_site/_ro/pypackages"):
    if _p not in sys.path:
        sys.path.insert(0, _p)

import math
import numpy as np
import ml_dtypes

BF16 = ml_dtypes.bfloat16

B = 2048
DIM = 384
KD = 16
HEADS = 8
DH = 512
OUT_DIM = 384
N = 49
EPS = 1e-5
SCALE = KD ** -0.5
NCORES = 8
NPC = B // NCORES            # elements per core
NB = 8                       # elements per tile
CCH = DIM // 128             # 3 c-chunks
VCH = DH // 128              # 4 hd-chunks
OCH = OUT_DIM // 128         # 3 o-chunks

# tap (r or s) -> valid output index range (i0, count): 2i + r - 1 in [0, 6]
def _dw_range(r):
    i0 = max(0, math.ceil((1 - r) / 2))
    i1 = min(3, (7 - r) // 2)
    return i0, i1 - i0 + 1


# center tap first: it covers the full 4x4 output, so it can initialize the
# accumulator (PSUM start=True for q; tensor_scalar write for v_local)
TAPS = [(1, 1)] + [(r, s) for r in range(3) for s in range(3) if (r, s) != (1, 1)]


def _bn_affine(p):
    g, be, m, v = (np.asarray(p[i], np.float64) for i in range(4))
    s = g / np.sqrt(v + EPS)
    t = be - m * s
    return s, t


def _fold(inputs):
    """Fold BNs/scale/pool into dense weights. Returns dict of host arrays."""
    f = lambda a: np.asarray(a, np.float64)
    qlw, qlb = f(inputs["qlw"]), f(inputs["qlb"])
    qpw, qpb = f(inputs["qpw"])[:, :, 0, 0], f(inputs["qpb"])
    kw = f(inputs["kw"])[:, :, 0, 0]
    vw, vb = f(inputs["vw"])[:, :, 0, 0], f(inputs["vb"])
    vlw, vlb = f(inputs["vlw"]), f(inputs["vlb"])
    pw, pb = f(inputs["pw"])[:, :, 0, 0], f(inputs["pb"])
    sq, tq = _bn_affine(inputs["qbn"])
    sk, _ = _bn_affine(inputs["kbn"])
    sv, tv = _bn_affine(inputs["vbn"])
    svl, tvl = _bn_affine(inputs["vlbn"])
    sp, tp = _bn_affine(inputs["pbn"])

    # Q path: q_out = SCALE * (sq*(qpw@q_in + qpb) + tq),  q_in = dw(x)+qlb+pool
    Wq = (SCALE * sq)[:, None] * qpw                      # [128, 384]
    qb_total = SCALE * (sq * qpb + tq) + Wq @ qlb         # [128]
    taps = [(Wq * qlw[:, 0, r, s][None, :]).T for (r, s) in TAPS]   # [384,128]
    taps.append((0.25 * Wq).T)                            # pool tap
    wq = np.stack(taps)                                   # [10, 384, 128]

    wk = (sk[:, None] * kw).T                             # [384, 128] lhsT
    wv = (sv[:, None] * vw).T                             # [384, 512] lhsT
    vb_t = sv * vb + tv                                   # [512]
    vlw_t = np.stack([svl * vlw[:, 0, r, s] for (r, s) in TAPS], axis=1)  # [512,9]
    vlb_t = svl * vlb + tvl                               # [512]
    wp = (sp[:, None] * pw).T                             # [512, 384] lhsT
    pb_t = sp * pb + tp                                   # [384]

    ab = np.asarray(inputs["ab"], np.float64)
    idx = np.asarray(inputs["bias_idxs"], np.int64)
    btab = ab[:, idx].reshape(HEADS * 16, N)              # [128, 49]

    mask = np.zeros((128, 128), np.float32)
    for h in range(HEADS):
        mask[h * 16:(h + 1) * 16, h * 16:(h + 1) * 16] = 1.0

    arrs = {}
    arrs["wk"] = np.stack([wk[ci * 128:(ci + 1) * 128, :] for ci in range(CCH)])
    arrs["wv"] = np.stack([wv[ci * 128:(ci + 1) * 128, o * 128:(o + 1) * 128]
                           for ci in range(CCH) for o in range(VCH)])
    arrs["wq"] = np.stack([wq[t, ci * 128:(ci + 1) * 128, :]
                           for t in range(10) for ci in range(CCH)])
    arrs["wp"] = np.stack([wp[c * 128:(c + 1) * 128, o * 128:(o + 1) * 128]
                           for c in range(VCH) for o in range(OCH)])
    for k in ("wk", "wv", "wq", "wp"):
        arrs[k] = np.ascontiguousarray(arrs[k]).astype(BF16)
    arrs["mask"] = mask.astype(BF16)
    arrs["btab"] = np.ascontiguousarray(btab).astype(np.float32)
    arrs["qb"] = qb_total.astype(np.float32).reshape(128, 1)
    arrs["vbc"] = np.ascontiguousarray(
        vb_t.astype(np.float32).reshape(VCH, 128).T)                  # [128, 4]
    arrs["pbc"] = np.ascontiguousarray(
        pb_t.astype(np.float32).reshape(OCH, 128).T)                  # [128, 3]
    arrs["vlw"] = np.ascontiguousarray(
        vlw_t.astype(np.float32).reshape(VCH, 128, 9).transpose(1, 0, 2))  # [128,4,9]
    wvl = np.zeros((9 * VCH, 128, 128), np.float32)
    for t in range(9):
        for c in range(VCH):
            np.fill_diagonal(wvl[t * VCH + c], vlw_t[c * 128:(c + 1) * 128, t])
    arrs["wvl"] = wvl.astype(BF16)
    arrs["vlb"] = np.ascontiguousarray(
        vlb_t.astype(np.float32).reshape(VCH, 128).T)                 # [128, 4]
    return arrs


# ----------------------------------------------------------------------------
# kernel builder
# ----------------------------------------------------------------------------

def _redim(ap, free_dims):
    """Keep an AP's tensor/offset/partition-dim (from slicing) but replace
    its free dims."""
    import concourse.bass as bass
    return bass.AP(tensor=ap.tensor, offset=ap.offset,
                   ap=[list(ap.ap[0])] + [list(d) for d in free_dims])


def _build(n_per_core):
    import concourse.bacc as bacc
    import concourse.tile as tile
    from concourse import mybir

    dt = mybir.dt
    AF = mybir.ActivationFunctionType
    OP = mybir.AluOpType
    ntiles = n_per_core // NB
    assert n_per_core % NB == 0

    nc = bacc.Bacc("TRN2", target_bir_lowering=False, debug=False,
                   num_devices=NCORES)

    xd = nc.dram_tensor("x", [n_per_core, DIM, N], dt.bfloat16,
                        kind="ExternalInput").ap()
    yd = nc.dram_tensor("y", [n_per_core, OUT_DIM, 16], dt.bfloat16,
                        kind="ExternalOutput").ap()
    wkd = nc.dram_tensor("wk", [3, 128, 128], dt.bfloat16, kind="ExternalInput").ap()
    wvd = nc.dram_tensor("wv", [12, 128, 128], dt.bfloat16, kind="ExternalInput").ap()
    wqd = nc.dram_tensor("wq", [30, 128, 128], dt.bfloat16, kind="ExternalInput").ap()
    wpd = nc.dram_tensor("wp", [12, 128, 128], dt.bfloat16, kind="ExternalInput").ap()
    maskd = nc.dram_tensor("mask", [128, 128], dt.bfloat16, kind="ExternalInput").ap()
    btabd = nc.dram_tensor("btab", [128, N], dt.float32, kind="ExternalInput").ap()
    qbd = nc.dram_tensor("qb", [128, 1], dt.float32, kind="ExternalInput").ap()
    vbcd = nc.dram_tensor("vbc", [128, VCH], dt.float32, kind="ExternalInput").ap()
    pbcd = nc.dram_tensor("pbc", [128, OCH], dt.float32, kind="ExternalInput").ap()
    wvld = nc.dram_tensor("wvl", [9 * VCH, 128, 128], dt.bfloat16,
                          kind="ExternalInput").ap()
    vlbd = nc.dram_tensor("vlb", [128, VCH], dt.float32, kind="ExternalInput").ap()

    with tile.TileContext(nc) as tc:
        with (
            tc.tile_pool(name="const", bufs=1) as constp,
            tc.tile_pool(name="pers", bufs=1) as persp,
            tc.tile_pool(name="xin", bufs=3) as xp,
            tc.tile_pool(name="work", bufs=2) as wkp,
            tc.tile_pool(name="tr", bufs=2) as trp,
            tc.tile_pool(name="ps", bufs=1, space="PSUM") as psp,
            tc.tile_pool(name="ps2", bufs=1, space="PSUM") as psp2,
        ):
            # ---- constants ----
            wk_t = constp.tile([128, 3, 128], dt.bfloat16)
            nc.sync.dma_start(wk_t[:], wkd.rearrange("k c o -> c k o"))
            wv_t = constp.tile([128, 12, 128], dt.bfloat16)
            nc.sync.dma_start(wv_t[:], wvd.rearrange("k c o -> c k o"))
            wq_t = constp.tile([128, 30, 128], dt.bfloat16)
            nc.sync.dma_start(wq_t[:], wqd.rearrange("k c o -> c k o"))
            wp_t = constp.tile([128, 12, 128], dt.bfloat16)
            nc.sync.dma_start(wp_t[:], wpd.rearrange("k c o -> c k o"))
            mask_t = constp.tile([128, 128], dt.bfloat16)
            nc.sync.dma_start(mask_t[:], maskd)
            btab_t = constp.tile([128, N], dt.float32)
            nc.sync.dma_start(btab_t[:], btabd)
            qb_t = constp.tile([128, 1], dt.float32)
            nc.sync.dma_start(qb_t[:], qbd)
            vbc_t = constp.tile([128, VCH], dt.float32)
            nc.sync.dma_start(vbc_t[:], vbcd)
            pbc_t = constp.tile([128, OCH], dt.float32)
            nc.sync.dma_start(pbc_t[:], pbcd)
            wvl_t = constp.tile([128, 9 * VCH, 128], dt.bfloat16)
            nc.sync.dma_start(wvl_t[:], wvld.rearrange("k c o -> c k o"))
            vlb_t = constp.tile([128, VCH], dt.float32)
            nc.sync.dma_start(vlb_t[:], vlbd)

            # ---- persistent zero-padded panels (pads written once) ----
            vw_t = [persp.tile([128, NB, 128], dt.bfloat16, tag=f"vwide{c}",
                               name=f"vwide{c}") for c in range(VCH)]
            pw_panel = persp.tile([128, NB, 128], dt.bfloat16, tag="pwide")
            xpad_t = [persp.tile([128, NB, 81], dt.bfloat16, tag=f"xpad{ci}",
                                 name=f"xpad{ci}") for ci in range(CCH)]
            xpool_t = [persp.tile([128, NB, 64], dt.bfloat16, tag=f"xpool{ci}",
                                  name=f"xpool{ci}") for ci in range(CCH)]
            for t in vw_t + xpad_t + xpool_t:
                nc.gpsimd.memset(t[:], 0.0)
            nc.gpsimd.memset(pw_panel[:], 0.0)

            dwr = {r: _dw_range(r) for r in range(3)}

            for it in range(ntiles):
                t0 = it * NB
                # ---------------- x in ----------------
                x_t = [xp.tile([128, NB, N], dt.bfloat16, tag=f"x{ci}",
                               name=f"xt{ci}") for ci in range(CCH)]
                for ci in range(CCH):
                    nc.sync.dma_start(
                        x_t[ci][:],
                        xd[t0:t0 + NB, ci * 128:(ci + 1) * 128, :]
                        .rearrange("b c n -> c b n"))

                # ---------------- K conv ----------------
                ps_k = psp.tile([128, NB, N], dt.float32, tag="psk")
                for ci in range(CCH):
                    nc.tensor.matmul(ps_k[:], wk_t[:, ci, :], x_t[ci][:],
                                     start=(ci == 0), stop=(ci == CCH - 1))
                k_sb = wkp.tile([128, NB, N], dt.bfloat16, tag="ksb")
                nc.vector.tensor_copy(k_sb[:], ps_k[:])

                # ---------------- Q taps ----------------
                # zero-padded panels: xpad = 9x9 (pad=1), xpool = 8x8 with
                # rows/cols >= 6 zeroed (emulates avgpool's zero-pad to 4x4)
                for ci in range(CCH):
                    nc.scalar.activation(
                        _redim(xpad_t[ci][:, :, 10:], [[81, NB], [9, 7], [1, 7]]),
                        x_t[ci][:].rearrange("p b (i j) -> p b i j", i=7),
                        AF.Identity)
                    nc.vector.tensor_copy(
                        _redim(xpool_t[ci][:], [[64, NB], [8, 6], [1, 6]]),
                        _redim(x_t[ci][:], [[N, NB], [7, 6], [1, 6]]))
                ps_qy = psp.tile([128, 4, NB, 16], dt.float32, tag="psqy")
                ps_q = ps_qy[:, 3]
                nmm = (len(TAPS) + 4) * CCH
                mm = 0
                for ti, (r, s) in enumerate(TAPS):
                    for ci in range(CCH):
                        rhs = _redim(xpad_t[ci][:, :, r * 9 + s:],
                                     [[81, NB], [18, 4], [2, 4]])
                        mm += 1
                        nc.tensor.matmul(ps_q, wq_t[:, ti * 3 + ci, :], rhs,
                                         start=(mm == 1), stop=False)
                for (u, v) in ((0, 0), (0, 1), (1, 0), (1, 1)):
                    for ci in range(CCH):
                        rhs = _redim(xpool_t[ci][:, :, u * 8 + v:],
                                     [[64, NB], [16, 4], [2, 4]])
                        mm += 1
                        nc.tensor.matmul(ps_q, wq_t[:, 9 * 3 + ci, :], rhs,
                                         start=False, stop=(mm == nmm))
                q_sb = wkp.tile([128, NB, 16], dt.bfloat16, tag="qsb")
                nc.scalar.activation(q_sb[:], ps_q, AF.Identity, bias=qb_t[:])

                # ---------------- V conv (two half-passes over hd) ----------
                for half in range(2):
                    ps_v = psp.tile([128, 2, 512], dt.float32, tag="psv")
                    for oo in range(2):
                        o = half * 2 + oo
                        for ci in range(CCH):
                            nc.tensor.matmul(
                                ps_v[:, oo, 0:NB * N], wv_t[:, ci * 4 + o, :],
                                x_t[ci][:], start=(ci == 0), stop=(ci == CCH - 1))
                    for oo in range(2):
                        o = half * 2 + oo
                        src = ps_v[:, oo, 0:NB * N].rearrange(
                            "p (b i j) -> p b i j", b=NB, i=7)
                        dst = _redim(vw_t[o][:, :, 10:],
                                     [[128, NB], [9, 7], [1, 7]])
                        nc.vector.tensor_scalar(dst, src,
                                                vbc_t[:, o:o + 1], None,
                                                op0=OP.add)

                # ------- v_local depthwise: diagonal-stationary matmuls -----
                ps_vl = psp.tile([128, VCH, NB, 16], dt.float32, tag="psvl")
                for c in range(VCH):
                    for ti, (r, s) in enumerate(TAPS):
                        rhs = _redim(vw_t[c][:, :, r * 9 + s:],
                                     [[128, NB], [18, 4], [2, 4]])
                        nc.tensor.matmul(ps_vl[:, c], wvl_t[:, ti * VCH + c, :],
                                         rhs, start=(ti == 0), stop=(ti == 8))

                # ---------------- q_bd + QK^T ----------------
                q_bd = wkp.tile([128, NB, 8, 16], dt.bfloat16, tag="qbd")
                in0 = _redim(q_sb[:], [[16, NB], [0, 8], [1, 16]])
                in1 = _redim(mask_t[:], [[0, NB], [16, 8], [1, 16]])
                nc.vector.tensor_tensor(q_bd[:], in0, in1, op=OP.mult)

                ps_a = psp.tile([128, NB, N], dt.float32, tag="psa")
                for e in range(NB):
                    nc.tensor.matmul(ps_a[:, e, :],
                                     q_bd[:, e].rearrange("p h m -> p (h m)"),
                                     k_sb[:, e, :], start=True, stop=True)

                # bias + softmax (no max subtraction; logits are small)
                bt = _redim(btab_t[:], [[0, NB], [1, N]])
                nc.vector.tensor_tensor(ps_a[:], ps_a[:], bt, op=OP.add)
                pint = _redim(pw_panel[:, :, 10:], [[128, NB], [9, 7], [1, 7]])
                nc.scalar.activation(pint,
                                     ps_a[:].rearrange("p b (i j) -> p b i j", i=7),
                                     AF.Exp)
                sums = wkp.tile([128, NB], dt.float32, tag="sums")
                nc.vector.reduce_sum(sums[:], pint, axis=mybir.AxisListType.XY)
                rs = wkp.tile([128, NB], dt.float32, tag="rs")
                nc.vector.reciprocal(rs[:], sums[:])
                rsb = _redim(rs[:], [[1, NB], [0, 7], [0, 7]])
                nc.vector.tensor_tensor(pint, pint, rsb, op=OP.mult)

                # ---------------- transposes (DMA xbar) ----------------
                pt = trp.tile([128, NB, 128], dt.bfloat16, tag="pt")
                for e in range(NB):
                    nc.sync.dma_start(pt[:, e, :], pw_panel[:, e, :],
                                      transpose=True)
                vt = trp.tile([128, NB, VCH, 128], dt.bfloat16, tag="vt")
                for e in range(NB):
                    for c in range(VCH):
                        nc.sync.dma_start(vt[:, e, c, :], vw_t[c][:, e, :],
                                          transpose=True)

                # ---------------- A @ V ----------------
                ps_xa = psp2.tile([128, VCH, NB, 32], dt.float32, tag="psxa")
                for e in range(NB):
                    for c in range(VCH):
                        nc.tensor.matmul(ps_xa[:, c, e, :], vt[:, e, c, :],
                                         pt[:, e, 32 * c:32 * (c + 1)],
                                         start=True, stop=True)

                # ---------------- xa + v_local, relu, proj ----------------
                vloc_sb = wkp.tile([128, VCH, NB, 16], dt.float32, tag="vlocsb")
                for c in range(VCH):
                    nc.scalar.activation(vloc_sb[:, c], ps_vl[:, c],
                                         AF.Identity, bias=vlb_t[:, c:c + 1])
                pre = wkp.tile([128, VCH, NB, 16], dt.bfloat16, tag="pre")
                for c in range(VCH):
                    for hh in range(2):
                        p0 = hh * 64
                        xa = ps_xa[p0:p0 + 64, c, :, hh * 16:(hh + 1) * 16]
                        vl = vloc_sb[p0:p0 + 64, c]
                        dst = pre[p0:p0 + 64, c]
                        nc.vector.scalar_tensor_tensor(
                            dst, xa, 1.0, vl, op0=OP.mult, op1=OP.add)
                relu = wkp.tile([128, VCH, NB, 16], dt.bfloat16, tag="relu")
                for c in range(VCH):
                    nc.scalar.activation(relu[:, c], pre[:, c], AF.Relu)

                ps_y = ps_qy
                for o in range(OCH):
                    for c in range(VCH):
                        nc.tensor.matmul(
                            ps_y[:, o],
                            wp_t[:, c * 3 + o, :],
                            relu[:, c].rearrange("p b m -> p (b m)"),
                            start=(c == 0), stop=(c == VCH - 1))
                y_sb = wkp.tile([128, OCH, NB, 16], dt.bfloat16, tag="ysb")
                for o in range(OCH):
                    nc.scalar.activation(y_sb[:, o], ps_y[:, o], AF.Identity,
                                         bias=pbc_t[:, o:o + 1])
                for o in range(OCH):
                    nc.sync.dma_start(
                        yd[t0:t0 + NB, o * 128:(o + 1) * 128, :]
                        .rearrange("b c m -> c b m"),
                        y_sb[:, o])

    nc.compile()
    return nc


# ----------------------------------------------------------------------------
# entry point
# ----------------------------------------------------------------------------

_CACHE = {}


def _get_nc(n_per_core):
    if n_per_core not in _CACHE:
        _CACHE[n_per_core] = _build(n_per_core)
    return _CACHE[n_per_core]


DRAM_INPUTS = ("wk", "wv", "wq", "wp", "wvl", "mask", "btab", "qb", "vbc",
               "pbc", "vlb")


def kernel(x, qlw, qlb, qpw, qpb, qbn, kw, kb, kbn, vw, vb, vbn,
           vlw, vlb, vlbn, pw, pb, pbn, ab, bias_idxs):
    from concourse.bass_utils import run_bass_kernel_spmd

    arrs = _fold(dict(qlw=qlw, qlb=qlb, qpw=qpw, qpb=qpb, qbn=qbn, kw=kw,
                      kb=kb, kbn=kbn, vw=vw, vb=vb, vbn=vbn, vlw=vlw,
                      vlb=vlb, vlbn=vlbn, pw=pw, pb=pb, pbn=pbn, ab=ab,
                      bias_idxs=bias_idxs))
    xb = np.ascontiguousarray(np.asarray(x, np.float32)).reshape(B, DIM, N)
    xb = xb.astype(BF16).reshape(NCORES, NPC, DIM, N)

    nc = _get_nc(NPC)
    in_maps = []
    for c in range(NCORES):
        m = {"x": np.ascontiguousarray(xb[c])}
        m.update({k: arrs[k] for k in DRAM_INPUTS})
        in_maps.append(m)
    res = run_bass_kernel_spmd(nc, in_maps, list(range(NCORES)))
    y = np.stack([res.results[c]["y"] for c in range(NCORES)])
    y = y.reshape(B, OUT_DIM, 4, 4).astype(np.float32)
    return y
